# revision 31
# baseline (speedup 1.0000x reference)
"""Trainium2 Bass kernel for nn_Attention_48498770706573.

Fused QKV-projection + masked softmax attention, sharded over 8 NeuronCores:
data-parallel over batch (B=2), tensor-parallel over heads (16 -> 4 per
core). Each core computes its (batch, 4-head) shard end to end; the host
only slices/transposes inputs (layout only, no arithmetic) and concatenates
the disjoint output shards.

Per-core dataflow (all "transposed-land"):
  qT/kT/vT [D, N] fp32 DRAM declared float32r (PE: 1 cycle/row vs fp32's 4),
  q/k projections in f32r -> qtT/ktT [256, N] stored bf16; v projection in
  bf16 -> vt [N, 256] bf16,
  scores S^T[nk, nq] as bf16 matmuls (separate LDWEIGHTS pipeline keeps the
  PE streaming) with two heads row-packed (K=64 each),
  exp on ACT straight out of PSUM -> bf16 (1/32 scale folded in),
  bool mask cast u8->bf16 on GpSimd, then one broadcast DVE multiply (2x
  bf16 mode) per scores group,
  PV with p^T (bf16) as moving operand, two heads col-packed per PSUM bank,
  row-sums via ones-column matmuls, 4 heads col-packed into one PSUM bank,
  V-bias as a rank-1 (bv x rowsum) matmul and normalization via a rank-1
  broadcast of 1/(rowsum+1e-6), then one DVE multiply per output tile.
"""

import os

import numpy as np

import concourse.bacc as bacc
import concourse.mybir as mybir
import concourse.tile as tile
from concourse.bass_utils import run_bass_kernel_spmd

B, NQ, NK, D, H = 2, 2048, 2048, 1024, 16
DH = D // H  # 64
N_CORES = 8
HPC = H // (N_CORES // B)  # heads per core = 4
JW = HPC * DH  # per-core projection width = 256
NKT = NK // 128  # 16 nk tiles
NCH = 4  # nq chunks
CHW = NQ // NCH  # 512
DT = 8  # contraction d-tiles

f32 = mybir.dt.float32
f32r = mybir.dt.float32r
bf16 = mybir.dt.bfloat16
u8 = mybir.dt.uint8


def _build():
    nc = bacc.Bacc(
        "TRN2", target_bir_lowering=False, debug=False, num_devices=N_CORES
    )

    qT = nc.dram_tensor("qT", [D, NQ], f32r, kind="ExternalInput")
    kT = nc.dram_tensor("kT", [D, NK], f32r, kind="ExternalInput")
    vT = nc.dram_tensor("vT", [D, NK], f32r, kind="ExternalInput")
    maskT = nc.dram_tensor("maskT", [NK, NQ], u8, kind="ExternalInput")
    wqT = nc.dram_tensor("wqT", [D, JW], f32r, kind="ExternalInput")
    wkT = nc.dram_tensor("wkT", [D, JW], f32r, kind="ExternalInput")
    wvT = nc.dram_tensor("wvT", [D, JW], f32r, kind="ExternalInput")
    bqd = nc.dram_tensor("bq", [2, 128], f32, kind="ExternalInput")
    bkd = nc.dram_tensor("bk", [2, 128], f32, kind="ExternalInput")
    # bv2[64p + 0, 0:64] = bv[128p + dh], bv2[64p + 32, 64:128] = bv[...]
    bvd = nc.dram_tensor("bv2", [128, 128], f32r, kind="ExternalInput")
    # ones2[64p, 0:64] = 1, ones2[64p + 32, 64:128] = 1
    onesd = nc.dram_tensor("ones2", [128, 128], f32r, kind="ExternalInput")
    onespd = nc.dram_tensor("onesp", [128, 32], bf16, kind="ExternalInput")
    o = nc.dram_tensor("o", [2 * 128, NQ], f32, kind="ExternalOutput")

    with tile.TileContext(nc) as tc:
        with (
            tc.tile_pool(name="consts", bufs=1) as consts,
            tc.tile_pool(name="wtmp", bufs=1) as wtmp,
            tc.tile_pool(name="stage", bufs=12) as stage,
            tc.tile_pool(name="vbfp", bufs=8) as vbfp,
            tc.tile_pool(name="xbfp", bufs=10) as xbfp,
            tc.tile_pool(name="qpool", bufs=12) as qpool,
            tc.tile_pool(name="m8pool", bufs=16) as m8pool,
            tc.tile_pool(name="mbpool", bufs=4) as mbpool,
            tc.tile_pool(name="projout", bufs=1) as projout,
            tc.tile_pool(name="ppool", bufs=5) as ppool,
            tc.tile_pool(name="rspool", bufs=2) as rspool,
            tc.tile_pool(name="outsb", bufs=3) as outsb,
            tc.tile_pool(name="sps", bufs=1, space="PSUM") as sps,
            tc.tile_pool(name="pvps", bufs=2, space="PSUM") as pvps,
            tc.tile_pool(name="rsps", bufs=2, space="PSUM") as rsps,
        ):
            # ---- constants ----
            w_sb = {}

            def dma_w(name, dram):
                t = wtmp.tile([128, DT, JW], f32r, tag=f"wt{name}", name="wt")
                for d in range(DT):
                    nc.sync.dma_start(
                        t[:, d], dram[d * 128 : (d + 1) * 128, :]
                    )
                return t

            def conv_w(name, t):
                wb = consts.tile([128, DT, JW], bf16, tag=f"w{name}", name="w")
                nc.vector.tensor_copy(wb, t)
                w_sb[name] = wb
            bq_sb = consts.tile([128, 2], f32, tag="bq")
            bk_sb = consts.tile([128, 2], f32, tag="bk")
            for m in range(2):
                nc.sync.dma_start(
                    bq_sb[:, m : m + 1],
                    bqd[m : m + 1, :].rearrange("a b -> b a"),
                )
                nc.sync.dma_start(
                    bk_sb[:, m : m + 1],
                    bkd[m : m + 1, :].rearrange("a b -> b a"),
                )
            bv_sb = consts.tile([128, 128], f32r, tag="bv")
            nc.sync.dma_start(bv_sb, bvd[:])
            ones_sb = consts.tile([128, 128], f32r, tag="ones")
            nc.sync.dma_start(ones_sb, onesd[:])
            onesp_sb = consts.tile([128, 32], bf16, tag="onesp")
            nc.sync.dma_start(onesp_sb, onespd[:])

            # ---- decoupled input DMAs (emitted in priority order) ----
            def dma_x_chunk(src, ch, tiles=None, pool=None):
                pool = pool or stage
                tiles = {} if tiles is None else tiles
                for d in range(DT):
                    x = pool.tile([128, CHW], f32r, tag="xc", name="x")
                    nc.sync.dma_start(
                        x,
                        src[d * 128 : (d + 1) * 128, ch * CHW : (ch + 1) * CHW],
                    )
                    tiles[(d, ch)] = x
                return tiles

            def dma_x(src):
                tiles = {}
                for d in range(DT):
                    for ch in range(NCH):
                        x = stage.tile([128, CHW], f32r, tag="xc", name="x")
                        nc.sync.dma_start(
                            x,
                            src[
                                d * 128 : (d + 1) * 128,
                                ch * CHW : (ch + 1) * CHW,
                            ],
                        )
                        tiles[(d, ch)] = x
                return tiles

            wtk = dma_w("k", wkT)
            conv_w("k", wtk)
            k_tiles = {}
            for ch in range(NCH):
                x = stage.tile([128, CHW], f32r, tag="xc", name="x")
                nc.sync.dma_start(x, kT[0:128, ch * CHW : (ch + 1) * CHW])
                k_tiles[(0, ch)] = x
            wtq = dma_w("q", wqT)
            wtv = dma_w("v", wvT)
            for d in range(1, DT):
                for ch in range(NCH):
                    x = stage.tile([128, CHW], f32r, tag="xc", name="x")
                    nc.sync.dma_start(
                        x, kT[d * 128 : (d + 1) * 128, ch * CHW : (ch + 1) * CHW]
                    )
                    k_tiles[(d, ch)] = x
            q_tiles = dma_x_chunk(qT, 0)
            m8 = []
            for t in range(NKT):
                mt8 = m8pool.tile([128, NQ], u8, tag="m8", name="m8")
                nc.sync.dma_start(mt8, maskT[t * 128 : (t + 1) * 128, :])
                m8.append(mt8)
            v_tiles = {}
            for ch in range(NCH):
                dma_x_chunk(vT, ch, v_tiles)
            q_later = {}
            for ch in range(1, NCH):
                dma_x_chunk(qT, ch, q_later, pool=qpool)

            # ---- projections ----
            qtT = projout.tile([128, 2, NQ], bf16, tag="qtT")
            ktT = projout.tile([128, 2, NK], bf16, tag="ktT")
            vt = projout.tile([128, NKT, JW], bf16, tag="vt")

            def proj_qk_full(name, tiles, dst, bias):
                """All 4 chunks, one weight load per (m, d): m0 accumulates
                into a 4-bank sps tile, m1 into four psum singles."""
                ps0t = sps.tile([128, 4 * CHW], f32, tag="s", name="ps0")
                ps0 = [
                    ps0t[:, ch * CHW : (ch + 1) * CHW] for ch in range(NCH)
                ]
                ps1 = [
                    pvps.tile([128, CHW], f32, tag="pvpst", name=f"psa{i}")
                    for i in range(2)
                ] + [
                    rsps.tile([128, CHW], f32, tag="rspst", name=f"psb{i}")
                    for i in range(2)
                ]
                for d in range(DT):
                    xb = {}
                    for ch in range(NCH):
                        xb[ch] = xbfp.tile(
                            [128, CHW], bf16, tag="xb", name="xb"
                        )
                        nc.vector.tensor_copy(xb[ch], tiles[(d, ch)])
                    for ch in range(NCH):
                        nc.tensor.matmul(
                            ps0[ch],
                            w_sb[name][:, d, 0:128],
                            xb[ch],
                            start=(d == 0),
                            stop=(d == DT - 1),
                        )
                    for ch in range(NCH):
                        nc.tensor.matmul(
                            ps1[ch],
                            w_sb[name][:, d, 128:256],
                            xb[ch],
                            start=(d == 0),
                            stop=(d == DT - 1),
                        )
                nc.vector.tensor_scalar_add(dst[:, 0, :], ps0t, bias[:, 0:1])
                for ch in range(NCH):
                    nc.vector.tensor_scalar_add(
                        dst[:, 1, ch * CHW : (ch + 1) * CHW],
                        ps1[ch],
                        bias[:, 1:2],
                    )

            def proj_qk_chunk(name, tiles, ch, dst, bias, cast_dma=False):
                xb = {}
                for d in range(DT):
                    xb[d] = xbfp.tile([128, CHW], bf16, tag="xb", name="xb")
                    if cast_dma:
                        nc.gpsimd.dma_start(xb[d], tiles[(d, ch)])
                    else:
                        nc.vector.tensor_copy(xb[d], tiles[(d, ch)])
                for m in range(2):
                    ps = rsps.tile([128, CHW], f32, tag="rspst", name="pps")
                    for d in range(DT):
                        nc.tensor.matmul(
                            ps,
                            w_sb[name][:, d, m * 128 : (m + 1) * 128],
                            xb[d],
                            start=(d == 0),
                            stop=(d == DT - 1),
                        )
                    nc.vector.tensor_scalar_add(
                        dst[:, m, ch * CHW : (ch + 1) * CHW],
                        ps,
                        bias[:, m : m + 1],
                    )

            proj_qk_full("k", k_tiles, ktT, bk_sb)
            conv_w("q", wtq)
            conv_w("v", wtv)
            wv_bf = w_sb["v"]
            proj_qk_chunk("q", q_tiles, 0, qtT, bq_sb)

            def proj_v():
                # bf16 x-tiles so the weight loads pipeline with the matmuls
                vbf = {}
                for n in range(NKT):
                    ch, nn_ = divmod(n, 4)
                    ps = pvps.tile([128, JW], f32, tag="pvpst", name="vps")
                    for d in range(DT):
                        if (d, ch) not in vbf:
                            xb = vbfp.tile(
                                [128, CHW], bf16, tag="vb", name="vb"
                            )
                            nc.vector.tensor_copy(xb, v_tiles[(d, ch)])
                            vbf[(d, ch)] = xb
                        nc.tensor.matmul(
                            ps,
                            vbf[(d, ch)][:, nn_ * 128 : (nn_ + 1) * 128],
                            wv_bf[:, d, :],
                            start=(d == 0),
                            stop=(d == DT - 1),
                        )
                    nc.vector.tensor_copy(vt[:, n, :], ps)

            # ---- attention ----
            def scores_group(t, cs, p_tiles):
                sp = sps.tile([128, 4 * CHW], f32, tag="s", name="sp")
                for pair in range(2):
                    for hh in range(2):
                        nc.tensor.matmul(
                            sp[
                                :,
                                (2 * pair + hh) * CHW : (2 * pair + hh + 1)
                                * CHW,
                            ],
                            ktT[
                                64 * hh : 64 * (hh + 1),
                                pair,
                                t * 128 : (t + 1) * 128,
                            ],
                            qtT[64 * hh : 64 * (hh + 1), pair, cs],
                            start=True,
                            stop=True,
                        )
                p = ppool.tile([128, 4 * CHW], bf16, tag="p", name="p")
                p_tiles[t] = p
                mb = mbpool.tile([128, CHW], bf16, tag="mb", name="mb")
                # SWDGE cast DMA u8 -> bf16 (frees GpSimd compute)
                nc.gpsimd.dma_start(mb, m8[t][:, cs])
                nc.scalar.activation(
                    out=p,
                    in_=sp,
                    func=mybir.ActivationFunctionType.Exp,
                    scale=1.0 / 32.0,
                )
                p4 = p.rearrange("p (h c) -> p h c", h=4)
                nc.vector.tensor_mul(
                    p4,
                    p4,
                    mb.rearrange("p (a c) -> p a c", a=1).to_broadcast(
                        (128, 4, CHW)
                    ),
                )

            def pv_t(t, p_tiles, pv_ps, rs_ps):
                st, sp_ = t == 0, t == NKT - 1
                p = p_tiles[t]
                for pair in range(2):
                    for hh in range(2):
                        nc.tensor.matmul(
                            pv_ps[pair][64 * hh : 64 * (hh + 1), :],
                            vt[
                                :,
                                t,
                                128 * pair + 64 * hh : 128 * pair
                                + 64 * (hh + 1),
                            ],
                            p[:, (2 * pair + hh) * CHW : (2 * pair + hh + 1) * CHW],
                            start=st,
                            stop=sp_,
                            tile_position=(0, 64 * hh),
                        )
                for pair in range(2):
                    for hh in range(2):
                        hg = 2 * pair + hh
                        nc.tensor.matmul(
                            rs_ps[32 * hg : 32 * hg + 32, :],
                            onesp_sb[:, 0:32],
                            p[:, (2 * pair + hh) * CHW : (2 * pair + hh + 1) * CHW],
                            start=st,
                            stop=sp_,
                            tile_position=(0, 32 * hg),
                        )

            def chunk_tail(cs, pv_ps, rs_ps):
                # rowsum -> +eps -> reciprocal (all 128 rows valid: the M=32
                # rowsum matmuls wrote 32 identical rows per head)
                rs_sb = rspool.tile([128, CHW], f32r, tag="rssb", name="rssb")
                nc.vector.tensor_scalar_add(rs_sb, rs_ps, 1e-6)
                rc_sb = rspool.tile([128, CHW], f32r, tag="rcsb", name="rcsb")
                with nc.allow_low_precision(reason="f32r for rank-1 matmul"):
                    nc.vector.reciprocal(rc_sb, rs_sb)
                for pair in range(2):
                    # pv += bv (x) rowsum   (rank-1 via K=64, rows 0 and 32)
                    nc.tensor.matmul(
                        pv_ps[pair],
                        bv_sb[64 * pair : 64 * (pair + 1), :],
                        rs_sb[64 * pair : 64 * (pair + 1), :],
                        start=False,
                        stop=True,
                    )
                    rb = rsps.tile([128, CHW], f32, tag="rspst", name="rb")
                    nc.tensor.matmul(
                        rb,
                        ones_sb[64 * pair : 64 * (pair + 1), :],
                        rc_sb[64 * pair : 64 * (pair + 1), :],
                        start=True,
                        stop=True,
                    )
                    rb_sb = outsb.tile([128, CHW], f32, tag="rbsb", name="rbsb")
                    nc.vector.tensor_copy(rb_sb, rb)
                    osb = outsb.tile([128, CHW], f32, tag="o", name="osb")
                    nc.vector.tensor_mul(osb, pv_ps[pair], rb_sb)
                    nc.sync.dma_start(o[128 * pair : 128 * (pair + 1), cs], osb)

            def new_pv_tiles():
                pv_ps = [
                    pvps.tile([128, CHW], f32, tag="pvpst", name=f"pv{i}")
                    for i in range(2)
                ]
                rs_ps = rsps.tile([128, CHW], f32, tag="rspst", name="rsps_t")
                return pv_ps, rs_ps

            proj_v()

            # all chunks fully interleaved; chunk c+1's q-projection is
            # emitted between chunk c's PV drain and its tail so the PE has
            # work while the rowsum->reciprocal DVE chain runs
            pending_tail = None
            for ch in range(NCH):
                cs = slice(ch * CHW, (ch + 1) * CHW)
                p_tiles = {}
                pv_ps, rs_ps = new_pv_tiles()
                LAG = 3
                for t in range(NKT + LAG):
                    if t < NKT:
                        scores_group(t, cs, p_tiles)
                    if t == 1 and pending_tail is not None:
                        pending_tail()
                        pending_tail = None
                    if t >= LAG:
                        pv_t(t - LAG, p_tiles, pv_ps, rs_ps)
                if ch + 1 < NCH:
                    proj_qk_chunk("q", q_later, ch + 1, qtT, bq_sb, cast_dma=True)

                def _tail(cs=cs, pv_ps=pv_ps, rs_ps=rs_ps):
                    chunk_tail(cs, pv_ps, rs_ps)

                pending_tail = _tail
            pending_tail()

    nc.compile()
    return nc


_NC = None


def _get_nc():
    global _NC
    if _NC is None:
        _NC = _build()
    return _NC


def _shard(inputs):
    import ml_dtypes

    q, k, v = inputs["q"], inputs["k"], inputs["v"]
    mask = inputs["mask"]
    Wq, bq, Wk, bk, Wv, bv = (
        inputs[n] for n in ("Wq", "bq", "Wk", "bk", "Wv", "bv")
    )
    qT = [np.ascontiguousarray(np.asarray(q[b], np.float32).T) for b in range(B)]
    kT = [np.ascontiguousarray(np.asarray(k[b], np.float32).T) for b in range(B)]
    vT = [np.ascontiguousarray(np.asarray(v[b], np.float32).T) for b in range(B)]
    mT = [
        np.ascontiguousarray(np.asarray(mask[b]).T).view(np.uint8)
        for b in range(B)
    ]
    ones2 = np.zeros((128, 128), np.float32)
    for p in range(2):
        ones2[64 * p, 0:64] = 1.0
        ones2[64 * p + 32, 64:128] = 1.0
    onesp = np.ones((128, 32), ml_dtypes.bfloat16)
    in_maps = []
    for c in range(N_CORES):
        b, jg = divmod(c, N_CORES // B)
        j0 = jg * JW
        bvs = np.asarray(bv, np.float32)[j0 : j0 + JW]
        bv2 = np.zeros((128, 128), np.float32)
        for p in range(2):
            bv2[64 * p, 0:64] = bvs[128 * p : 128 * p + 64]
            bv2[64 * p + 32, 64:128] = bvs[128 * p + 64 : 128 * p + 128]
        in_maps.append(
            {
                "qT": qT[b],
                "kT": kT[b],
                "vT": vT[b],
                "maskT": mT[b],
                "wqT": np.ascontiguousarray(
                    np.asarray(Wq, np.float32)[j0 : j0 + JW, :].T
                ),
                "wkT": np.ascontiguousarray(
                    np.asarray(Wk, np.float32)[j0 : j0 + JW, :].T
                ),
                "wvT": np.ascontiguousarray(
                    np.asarray(Wv, np.float32)[j0 : j0 + JW, :].T
                ),
                "bq": np.asarray(bq, np.float32)[j0 : j0 + JW].reshape(2, 128),
                "bk": np.asarray(bk, np.float32)[j0 : j0 + JW].reshape(2, 128),
                "bv2": bv2,
                "ones2": ones2,
                "onesp": onesp,
            }
        )
    return in_maps


LAST_RESULT = None


def kernel(**inputs) -> np.ndarray:
    global LAST_RESULT
    nc = _get_nc()
    in_maps = _shard(inputs)
    trace = bool(int(os.environ.get("KTRACE", "0")))
    res = run_bass_kernel_spmd(
        nc,
        in_maps,
        core_ids=list(range(N_CORES)),
        trace=trace,
        trace_cores=[0] if trace else None,
    )
    LAST_RESULT = res
    out = np.empty((B, NQ, D), np.float32)
    for c in range(N_CORES):
        b, jg = divmod(c, N_CORES // B)
        j0 = jg * JW
        oc = res.results[c]["o"]  # [256, NQ] pair-major
        out[b, :, j0 : j0 + JW] = (
            oc.reshape(2, 2, DH, NQ).transpose(3, 0, 1, 2).reshape(NQ, JW)
        )
    return out


if __name__ == "__main__":
    if os.environ.get("KBUILD_ONLY"):
        import tempfile

        from concourse.bass_utils import compile_bass_kernel

        nc = _build()
        with tempfile.TemporaryDirectory() as td:
            compile_bass_kernel(nc, td)
        print("BUILD+COMPILE OK")


# revision 32
# speedup vs baseline: 1.0863x; 1.0863x over previous
"""Trainium2 Bass kernel for nn_Attention_48498770706573.

Fused QKV-projection + masked softmax attention, sharded over 8 NeuronCores:
data-parallel over batch (B=2), tensor-parallel over heads (16 -> 4 per
core). Each core computes its (batch, 4-head) shard end to end; the host
only slices/transposes inputs (layout only, no arithmetic) and concatenates
the disjoint output shards.

Per-core dataflow (all "transposed-land"):
  qT/kT/vT [D, N] fp32 DRAM declared float32r (PE: 1 cycle/row vs fp32's 4),
  q/k projections in f32r -> qtT/ktT [256, N] stored bf16; v projection in
  bf16 -> vt [N, 256] bf16,
  scores S^T[nk, nq] as bf16 matmuls (separate LDWEIGHTS pipeline keeps the
  PE streaming) with two heads row-packed (K=64 each),
  exp on ACT straight out of PSUM -> bf16 (1/32 scale folded in),
  bool mask cast u8->bf16 on GpSimd, then one broadcast DVE multiply (2x
  bf16 mode) per scores group,
  PV with p^T (bf16) as moving operand, two heads col-packed per PSUM bank,
  row-sums via ones-column matmuls, 4 heads col-packed into one PSUM bank,
  V-bias as a rank-1 (bv x rowsum) matmul and normalization via a rank-1
  broadcast of 1/(rowsum+1e-6), then one DVE multiply per output tile.
"""

import os

import numpy as np

import concourse.bacc as bacc
import concourse.mybir as mybir
import concourse.tile as tile
from concourse.bass_utils import run_bass_kernel_spmd

B, NQ, NK, D, H = 2, 2048, 2048, 1024, 16
DH = D // H  # 64
N_CORES = 8
HPC = H // (N_CORES // B)  # heads per core = 4
JW = HPC * DH  # per-core projection width = 256
NKT = NK // 128  # 16 nk tiles
NCH = 4  # nq chunks
CHW = NQ // NCH  # 512
DT = 8  # contraction d-tiles

f32 = mybir.dt.float32
f32r = mybir.dt.float32r
bf16 = mybir.dt.bfloat16
u8 = mybir.dt.uint8


def _build():
    nc = bacc.Bacc(
        "TRN2", target_bir_lowering=False, debug=False, num_devices=N_CORES
    )

    qT = nc.dram_tensor("qT", [D, NQ], f32r, kind="ExternalInput")
    kT = nc.dram_tensor("kT", [D, NK], f32r, kind="ExternalInput")
    vT = nc.dram_tensor("vT", [D, NK], f32r, kind="ExternalInput")
    maskT = nc.dram_tensor("maskT", [NK, NQ], u8, kind="ExternalInput")
    wqT = nc.dram_tensor("wqT", [D, JW], f32r, kind="ExternalInput")
    wkT = nc.dram_tensor("wkT", [D, JW], f32r, kind="ExternalInput")
    wvT = nc.dram_tensor("wvT", [D, JW], f32r, kind="ExternalInput")
    bqd = nc.dram_tensor("bq", [2, 128], f32, kind="ExternalInput")
    bkd = nc.dram_tensor("bk", [2, 128], f32, kind="ExternalInput")
    # bv2[64p + 0, 0:64] = bv[128p + dh], bv2[64p + 32, 64:128] = bv[...]
    bvd = nc.dram_tensor("bv2", [128, 128], f32r, kind="ExternalInput")
    # ones2[64p, 0:64] = 1, ones2[64p + 32, 64:128] = 1
    onesd = nc.dram_tensor("ones2", [128, 128], f32r, kind="ExternalInput")
    onespd = nc.dram_tensor("onesp", [128, 32], bf16, kind="ExternalInput")
    o = nc.dram_tensor("o", [2 * 128, NQ], f32, kind="ExternalOutput")

    with tile.TileContext(nc) as tc:
        with (
            tc.tile_pool(name="consts", bufs=1) as consts,
            tc.tile_pool(name="wtmp", bufs=1) as wtmp,
            tc.tile_pool(name="stage", bufs=12) as stage,
            tc.tile_pool(name="vbfp", bufs=8) as vbfp,
            tc.tile_pool(name="xbfp", bufs=10) as xbfp,
            tc.tile_pool(name="qpool", bufs=12) as qpool,
            tc.tile_pool(name="m8pool", bufs=16) as m8pool,
            tc.tile_pool(name="mbpool", bufs=4) as mbpool,
            tc.tile_pool(name="projout", bufs=1) as projout,
            tc.tile_pool(name="ppool", bufs=5) as ppool,
            tc.tile_pool(name="rspool", bufs=2) as rspool,
            tc.tile_pool(name="outsb", bufs=3) as outsb,
            tc.tile_pool(name="sps", bufs=2, space="PSUM") as sps,
            tc.tile_pool(name="pvps", bufs=2, space="PSUM") as pvps,
            tc.tile_pool(name="rsps", bufs=2, space="PSUM") as rsps,
        ):
            # ---- constants ----
            w_sb = {}

            def dma_w(name, dram):
                t = wtmp.tile([128, DT, JW], f32r, tag=f"wt{name}", name="wt")
                for d in range(DT):
                    nc.sync.dma_start(
                        t[:, d], dram[d * 128 : (d + 1) * 128, :]
                    )
                return t

            def conv_w(name, t):
                wb = consts.tile([128, DT, JW], bf16, tag=f"w{name}", name="w")
                nc.vector.tensor_copy(wb, t)
                w_sb[name] = wb
            bq_sb = consts.tile([128, 2], f32, tag="bq")
            bk_sb = consts.tile([128, 2], f32, tag="bk")
            for m in range(2):
                nc.sync.dma_start(
                    bq_sb[:, m : m + 1],
                    bqd[m : m + 1, :].rearrange("a b -> b a"),
                )
                nc.sync.dma_start(
                    bk_sb[:, m : m + 1],
                    bkd[m : m + 1, :].rearrange("a b -> b a"),
                )
            bv_sb = consts.tile([128, 128], f32r, tag="bv")
            nc.sync.dma_start(bv_sb, bvd[:])
            ones_sb = consts.tile([128, 128], f32r, tag="ones")
            nc.sync.dma_start(ones_sb, onesd[:])
            onesp_sb = consts.tile([128, 32], bf16, tag="onesp")
            nc.sync.dma_start(onesp_sb, onespd[:])

            # ---- decoupled input DMAs (emitted in priority order) ----
            def dma_x_chunk(src, ch, tiles=None, pool=None):
                pool = pool or stage
                tiles = {} if tiles is None else tiles
                for d in range(DT):
                    x = pool.tile([128, CHW], f32r, tag="xc", name="x")
                    nc.sync.dma_start(
                        x,
                        src[d * 128 : (d + 1) * 128, ch * CHW : (ch + 1) * CHW],
                    )
                    tiles[(d, ch)] = x
                return tiles

            def dma_x(src):
                tiles = {}
                for d in range(DT):
                    for ch in range(NCH):
                        x = stage.tile([128, CHW], f32r, tag="xc", name="x")
                        nc.sync.dma_start(
                            x,
                            src[
                                d * 128 : (d + 1) * 128,
                                ch * CHW : (ch + 1) * CHW,
                            ],
                        )
                        tiles[(d, ch)] = x
                return tiles

            wtk = dma_w("k", wkT)
            conv_w("k", wtk)
            k_tiles = {}
            for ch in range(NCH):
                x = stage.tile([128, CHW], f32r, tag="xc", name="x")
                nc.sync.dma_start(x, kT[0:128, ch * CHW : (ch + 1) * CHW])
                k_tiles[(0, ch)] = x
            wtq = dma_w("q", wqT)
            wtv = dma_w("v", wvT)
            for d in range(1, DT):
                for ch in range(NCH):
                    x = stage.tile([128, CHW], f32r, tag="xc", name="x")
                    nc.sync.dma_start(
                        x, kT[d * 128 : (d + 1) * 128, ch * CHW : (ch + 1) * CHW]
                    )
                    k_tiles[(d, ch)] = x
            q_tiles = dma_x_chunk(qT, 0)
            m8 = []
            for t in range(NKT):
                mt8 = m8pool.tile([128, NQ], u8, tag="m8", name="m8")
                nc.sync.dma_start(mt8, maskT[t * 128 : (t + 1) * 128, :])
                m8.append(mt8)
            v_tiles = {}
            for ch in range(NCH):
                dma_x_chunk(vT, ch, v_tiles)
            q_later = {}
            for ch in range(1, NCH):
                dma_x_chunk(qT, ch, q_later, pool=qpool)

            # ---- projections ----
            qtT = projout.tile([128, 2, NQ], bf16, tag="qtT")
            ktT = projout.tile([128, 2, NK], bf16, tag="ktT")
            vt = projout.tile([128, NKT, JW], bf16, tag="vt")

            def proj_qk_full(name, tiles, dst, bias):
                """All 4 chunks, one weight load per (m, d): m0 accumulates
                into a 4-bank sps tile, m1 into four psum singles."""
                ps0t = [
                    sps.tile([128, 2 * CHW], f32, tag="s", name=f"ps0{i}")
                    for i in range(2)
                ]
                ps0 = [
                    ps0t[0][:, 0:CHW],
                    ps0t[0][:, CHW:],
                    ps0t[1][:, 0:CHW],
                    ps0t[1][:, CHW:],
                ]
                ps1 = [
                    pvps.tile([128, CHW], f32, tag="pvpst", name=f"psa{i}")
                    for i in range(2)
                ] + [
                    rsps.tile([128, CHW], f32, tag="rspst", name=f"psb{i}")
                    for i in range(2)
                ]
                for d in range(DT):
                    xb = {}
                    for ch in range(NCH):
                        xb[ch] = xbfp.tile(
                            [128, CHW], bf16, tag="xb", name="xb"
                        )
                        nc.vector.tensor_copy(xb[ch], tiles[(d, ch)])
                    for ch in range(NCH):
                        nc.tensor.matmul(
                            ps0[ch],
                            w_sb[name][:, d, 0:128],
                            xb[ch],
                            start=(d == 0),
                            stop=(d == DT - 1),
                        )
                    for ch in range(NCH):
                        nc.tensor.matmul(
                            ps1[ch],
                            w_sb[name][:, d, 128:256],
                            xb[ch],
                            start=(d == 0),
                            stop=(d == DT - 1),
                        )
                for ch2 in range(2):
                    nc.vector.tensor_scalar_add(
                        dst[:, 0, ch2 * 2 * CHW : (ch2 + 1) * 2 * CHW],
                        ps0t[ch2],
                        bias[:, 0:1],
                    )
                for ch in range(NCH):
                    nc.vector.tensor_scalar_add(
                        dst[:, 1, ch * CHW : (ch + 1) * CHW],
                        ps1[ch],
                        bias[:, 1:2],
                    )

            def proj_qk_chunk(name, tiles, ch, dst, bias, cast_dma=False):
                xb = {}
                for d in range(DT):
                    xb[d] = xbfp.tile([128, CHW], bf16, tag="xb", name="xb")
                    if cast_dma:
                        nc.gpsimd.dma_start(xb[d], tiles[(d, ch)])
                    else:
                        nc.vector.tensor_copy(xb[d], tiles[(d, ch)])
                for m in range(2):
                    ps = rsps.tile([128, CHW], f32, tag="rspst", name="pps")
                    for d in range(DT):
                        nc.tensor.matmul(
                            ps,
                            w_sb[name][:, d, m * 128 : (m + 1) * 128],
                            xb[d],
                            start=(d == 0),
                            stop=(d == DT - 1),
                        )
                    nc.vector.tensor_scalar_add(
                        dst[:, m, ch * CHW : (ch + 1) * CHW],
                        ps,
                        bias[:, m : m + 1],
                    )

            proj_qk_full("k", k_tiles, ktT, bk_sb)
            conv_w("q", wtq)
            conv_w("v", wtv)
            wv_bf = w_sb["v"]
            proj_qk_chunk("q", q_tiles, 0, qtT, bq_sb)

            def proj_v():
                # bf16 x-tiles so the weight loads pipeline with the matmuls
                vbf = {}
                for n in range(NKT):
                    ch, nn_ = divmod(n, 4)
                    ps = pvps.tile([128, JW], f32, tag="pvpst", name="vps")
                    for d in range(DT):
                        if (d, ch) not in vbf:
                            xb = vbfp.tile(
                                [128, CHW], bf16, tag="vb", name="vb"
                            )
                            nc.vector.tensor_copy(xb, v_tiles[(d, ch)])
                            vbf[(d, ch)] = xb
                        nc.tensor.matmul(
                            ps,
                            vbf[(d, ch)][:, nn_ * 128 : (nn_ + 1) * 128],
                            wv_bf[:, d, :],
                            start=(d == 0),
                            stop=(d == DT - 1),
                        )
                    nc.vector.tensor_copy(vt[:, n, :], ps)

            # ---- attention ----
            def scores_group(pair, t, cs, p_tiles):
                sp = sps.tile([128, 2 * CHW], f32, tag="s", name="sp")
                for hh in range(2):
                    nc.tensor.matmul(
                        sp[:, hh * CHW : (hh + 1) * CHW],
                        ktT[
                            64 * hh : 64 * (hh + 1),
                            pair,
                            t * 128 : (t + 1) * 128,
                        ],
                        qtT[64 * hh : 64 * (hh + 1), pair, cs],
                        start=True,
                        stop=True,
                    )
                if pair == 0:
                    p = ppool.tile([128, 4 * CHW], bf16, tag="p", name="p")
                    p_tiles[t] = p
                    mb = mbpool.tile([128, CHW], bf16, tag="mb", name="mb")
                    # SWDGE cast DMA u8 -> bf16 (frees GpSimd compute)
                    nc.gpsimd.dma_start(mb, m8[t][:, cs])
                    p_tiles[("mb", t)] = mb
                else:
                    p = p_tiles[t]
                nc.scalar.activation(
                    out=p[:, 2 * pair * CHW : (2 * pair + 2) * CHW],
                    in_=sp,
                    func=mybir.ActivationFunctionType.Exp,
                    scale=1.0 / 32.0,
                )
                if pair == 1:
                    mb = p_tiles[("mb", t)]
                    p4 = p.rearrange("p (h c) -> p h c", h=4)
                    nc.vector.tensor_mul(
                        p4,
                        p4,
                        mb.rearrange("p (a c) -> p a c", a=1).to_broadcast(
                            (128, 4, CHW)
                        ),
                    )

            def pv_t(t, p_tiles, pv_ps, rs_ps):
                st, sp_ = t == 0, t == NKT - 1
                p = p_tiles[t]
                for pair in range(2):
                    for hh in range(2):
                        nc.tensor.matmul(
                            pv_ps[pair][64 * hh : 64 * (hh + 1), :],
                            vt[
                                :,
                                t,
                                128 * pair + 64 * hh : 128 * pair
                                + 64 * (hh + 1),
                            ],
                            p[:, (2 * pair + hh) * CHW : (2 * pair + hh + 1) * CHW],
                            start=st,
                            stop=sp_,
                            tile_position=(0, 64 * hh),
                        )
                for pair in range(2):
                    for hh in range(2):
                        hg = 2 * pair + hh
                        nc.tensor.matmul(
                            rs_ps[32 * hg : 32 * hg + 32, :],
                            onesp_sb[:, 0:32],
                            p[:, (2 * pair + hh) * CHW : (2 * pair + hh + 1) * CHW],
                            start=st,
                            stop=sp_,
                            tile_position=(0, 32 * hg),
                        )

            def chunk_tail(cs, pv_ps, rs_ps):
                # rowsum -> +eps -> reciprocal (all 128 rows valid: the M=32
                # rowsum matmuls wrote 32 identical rows per head)
                rs_sb = rspool.tile([128, CHW], f32r, tag="rssb", name="rssb")
                nc.vector.tensor_scalar_add(rs_sb, rs_ps, 1e-6)
                rc_sb = rspool.tile([128, CHW], f32r, tag="rcsb", name="rcsb")
                with nc.allow_low_precision(reason="f32r for rank-1 matmul"):
                    nc.vector.reciprocal(rc_sb, rs_sb)
                for pair in range(2):
                    # pv += bv (x) rowsum   (rank-1 via K=64, rows 0 and 32)
                    nc.tensor.matmul(
                        pv_ps[pair],
                        bv_sb[64 * pair : 64 * (pair + 1), :],
                        rs_sb[64 * pair : 64 * (pair + 1), :],
                        start=False,
                        stop=True,
                    )
                    rb = rsps.tile([128, CHW], f32, tag="rspst", name="rb")
                    nc.tensor.matmul(
                        rb,
                        ones_sb[64 * pair : 64 * (pair + 1), :],
                        rc_sb[64 * pair : 64 * (pair + 1), :],
                        start=True,
                        stop=True,
                    )
                    rb_sb = outsb.tile([128, CHW], f32, tag="rbsb", name="rbsb")
                    nc.vector.tensor_copy(rb_sb, rb)
                    osb = outsb.tile([128, CHW], f32, tag="o", name="osb")
                    nc.vector.tensor_mul(osb, pv_ps[pair], rb_sb)
                    nc.sync.dma_start(o[128 * pair : 128 * (pair + 1), cs], osb)

            def new_pv_tiles():
                pv_ps = [
                    pvps.tile([128, CHW], f32, tag="pvpst", name=f"pv{i}")
                    for i in range(2)
                ]
                rs_ps = rsps.tile([128, CHW], f32, tag="rspst", name="rsps_t")
                return pv_ps, rs_ps

            proj_v()

            # all chunks fully interleaved; chunk c+1's q-projection is
            # emitted between chunk c's PV drain and its tail so the PE has
            # work while the rowsum->reciprocal DVE chain runs
            pending_tail = None
            for ch in range(NCH):
                cs = slice(ch * CHW, (ch + 1) * CHW)
                p_tiles = {}
                pv_ps, rs_ps = new_pv_tiles()
                LAG = 3
                for t in range(NKT + LAG):
                    if t < NKT:
                        for pair in range(2):
                            scores_group(pair, t, cs, p_tiles)
                    if t == 1 and pending_tail is not None:
                        pending_tail()
                        pending_tail = None
                    if t >= LAG:
                        pv_t(t - LAG, p_tiles, pv_ps, rs_ps)
                if ch + 1 < NCH:
                    proj_qk_chunk("q", q_later, ch + 1, qtT, bq_sb, cast_dma=True)

                def _tail(cs=cs, pv_ps=pv_ps, rs_ps=rs_ps):
                    chunk_tail(cs, pv_ps, rs_ps)

                pending_tail = _tail
            pending_tail()

    nc.compile()
    return nc


_NC = None


def _get_nc():
    global _NC
    if _NC is None:
        _NC = _build()
    return _NC


def _shard(inputs):
    import ml_dtypes

    q, k, v = inputs["q"], inputs["k"], inputs["v"]
    mask = inputs["mask"]
    Wq, bq, Wk, bk, Wv, bv = (
        inputs[n] for n in ("Wq", "bq", "Wk", "bk", "Wv", "bv")
    )
    qT = [np.ascontiguousarray(np.asarray(q[b], np.float32).T) for b in range(B)]
    kT = [np.ascontiguousarray(np.asarray(k[b], np.float32).T) for b in range(B)]
    vT = [np.ascontiguousarray(np.asarray(v[b], np.float32).T) for b in range(B)]
    mT = [
        np.ascontiguousarray(np.asarray(mask[b]).T).view(np.uint8)
        for b in range(B)
    ]
    ones2 = np.zeros((128, 128), np.float32)
    for p in range(2):
        ones2[64 * p, 0:64] = 1.0
        ones2[64 * p + 32, 64:128] = 1.0
    onesp = np.ones((128, 32), ml_dtypes.bfloat16)
    in_maps = []
    for c in range(N_CORES):
        b, jg = divmod(c, N_CORES // B)
        j0 = jg * JW
        bvs = np.asarray(bv, np.float32)[j0 : j0 + JW]
        bv2 = np.zeros((128, 128), np.float32)
        for p in range(2):
            bv2[64 * p, 0:64] = bvs[128 * p : 128 * p + 64]
            bv2[64 * p + 32, 64:128] = bvs[128 * p + 64 : 128 * p + 128]
        in_maps.append(
            {
                "qT": qT[b],
                "kT": kT[b],
                "vT": vT[b],
                "maskT": mT[b],
                "wqT": np.ascontiguousarray(
                    np.asarray(Wq, np.float32)[j0 : j0 + JW, :].T
                ),
                "wkT": np.ascontiguousarray(
                    np.asarray(Wk, np.float32)[j0 : j0 + JW, :].T
                ),
                "wvT": np.ascontiguousarray(
                    np.asarray(Wv, np.float32)[j0 : j0 + JW, :].T
                ),
                "bq": np.asarray(bq, np.float32)[j0 : j0 + JW].reshape(2, 128),
                "bk": np.asarray(bk, np.float32)[j0 : j0 + JW].reshape(2, 128),
                "bv2": bv2,
                "ones2": ones2,
                "onesp": onesp,
            }
        )
    return in_maps


LAST_RESULT = None


def kernel(**inputs) -> np.ndarray:
    global LAST_RESULT
    nc = _get_nc()
    in_maps = _shard(inputs)
    trace = bool(int(os.environ.get("KTRACE", "0")))
    res = run_bass_kernel_spmd(
        nc,
        in_maps,
        core_ids=list(range(N_CORES)),
        trace=trace,
        trace_cores=[0] if trace else None,
    )
    LAST_RESULT = res
    out = np.empty((B, NQ, D), np.float32)
    for c in range(N_CORES):
        b, jg = divmod(c, N_CORES // B)
        j0 = jg * JW
        oc = res.results[c]["o"]  # [256, NQ] pair-major
        out[b, :, j0 : j0 + JW] = (
            oc.reshape(2, 2, DH, NQ).transpose(3, 0, 1, 2).reshape(NQ, JW)
        )
    return out


if __name__ == "__main__":
    if os.environ.get("KBUILD_ONLY"):
        import tempfile

        from concourse.bass_utils import compile_bass_kernel

        nc = _build()
        with tempfile.TemporaryDirectory() as td:
            compile_bass_kernel(nc, td)
        print("BUILD+COMPILE OK")
